# revision 28
# baseline (speedup 1.0000x reference)
"""AdaptiveFeaturePooling (2-level FPN ROI-align, adaptive sampling, summed)
as a Trainium2 Bass kernel on 8 NeuronCores.

Strategy
--------
The reference roi_align (sampling_ratio=-1, aligned=False, smax=2) is exactly
separable per ROI and level:

    out[r,c,py,px] = sum_lvl sum_{y,x} Wy[r,py,y] * Wx[r,px,x] * feat[b_r,c,y,x]

with Wy/Wx computable from boxes alone (1/count folded in).  The host
precomputes, for every ROI, the list of contributing feature pixels (both
levels concatenated) and the dense weight matrix
W2[(y,x), (py,px)] = Wy[py,y]*Wx[px,x], splits the pixel list into K=128
chunks, and packs pixel-value rows + weight rows into dense bf16 arrays.

Each ROI on device is then a PSUM-accumulated chain of matmuls:

    psum[196(=2x98), 256c] += W2_chunk[128,196].T @ pixvals_chunk[128,256]

ROIs are sharded across the 8 cores round-robin by descending chunk count and
padded so every core executes an identical instruction stream (SPMD).  Output
[196, 256] per ROI is copied to SBUF as bf16, DMA'd out, and the host does the
final [R,196,C] -> [R,C,14,14] layout fix + f32 cast while unsharding.
"""

import numpy as np
import ml_dtypes

P_OUT = 14
S_MAX = 2
N_CORES = 8
R_TOTAL = 512
C_FEAT = 256
CHUNK_K = 128
LEVELS = ((128, 0.25), (64, 0.125))  # (feature size, spatial_scale)

BF16_NP = ml_dtypes.bfloat16


# ---------------------------------------------------------------- host math
def _axis_weights(lo, hi, size):
    """Separable 1D ROI-align weights for one axis of one level.

    lo/hi: f32 [R] box edges in feature coords. Returns (W [R,14,size] f32
    with 1/g folded in, support lo index [R], support hi index [R])."""
    roi = np.maximum(hi - lo, np.float32(1.0))
    bin_ = roi / np.float32(P_OUT)
    g = np.ceil(roi / P_OUT).astype(np.int32)
    gf = g.astype(np.float32)
    p = np.arange(P_OUT, dtype=np.float32)
    s = np.arange(S_MAX, dtype=np.float32)
    coord = (
        lo[:, None, None]
        + p[None, :, None] * bin_[:, None, None]
        + (s[None, None, :] + np.float32(0.5)) * (bin_ / gf)[:, None, None]
    )
    smask = np.arange(S_MAX)[None, :] < g[:, None]
    valid = (coord >= np.float32(-1.0)) & (coord <= np.float32(size))
    m = smask[:, None, :] & valid
    cc = np.clip(coord, np.float32(0.0), np.float32(size - 1))
    c0 = np.floor(cc).astype(np.int32)
    c1 = np.minimum(c0 + 1, size - 1)
    l = cc - c0.astype(np.float32)
    h = np.float32(1.0) - l
    wl = np.where(m, h, np.float32(0.0))
    wh = np.where(m, l, np.float32(0.0))
    R = lo.shape[0]
    W = np.zeros((R, P_OUT, size), np.float32)
    ridx = np.arange(R)[:, None, None]
    pidx = np.arange(P_OUT)[None, :, None]
    np.add.at(W, (ridx, pidx, c0), wl)
    np.add.at(W, (ridx, pidx, c1), wh)
    W /= gf[:, None, None]
    any_col = W.any(axis=1)
    has = any_col.any(axis=1)
    lo_i = np.where(has, np.argmax(any_col, axis=1), 0).astype(np.int64)
    hi_i = np.where(has, size - 1 - np.argmax(any_col[:, ::-1], axis=1), 0).astype(
        np.int64
    )
    return W, lo_i, hi_i


def _build_rois(boxes, batch_idx):
    """Per-ROI concatenated (level, y, x) pixel lists + W2 [K,196] f32."""
    per_level = []
    for size, scale in LEVELS:
        b = boxes * np.float32(scale)
        Wx, xlo, xhi = _axis_weights(b[:, 0], b[:, 2], size)
        Wy, ylo, yhi = _axis_weights(b[:, 1], b[:, 3], size)
        per_level.append((Wy, ylo, yhi, Wx, xlo, xhi))

    rois = []
    for r in range(boxes.shape[0]):
        lvls, ys_l, xs_l, wys, wxs = [], [], [], [], []
        for lvl, (Wy, ylo, yhi, Wx, xlo, xhi) in enumerate(per_level):
            ys = np.arange(ylo[r], yhi[r] + 1)
            xs = np.arange(xlo[r], xhi[r] + 1)
            if ys.size == 0 or xs.size == 0:
                continue
            WyS = Wy[r][:, ys]  # [14, Hr]
            WxS = Wx[r][:, xs]  # [14, Wr]
            yy, xx = np.meshgrid(ys, xs, indexing="ij")
            lvls.append(np.full(yy.size, lvl, np.int64))
            ys_l.append(yy.ravel())
            xs_l.append(xx.ravel())
            # per-pixel weight row vectors; device forms the outer product
            yloc = (yy - ys[0]).ravel()
            xloc = (xx - xs[0]).ravel()
            wys.append(WyS.T[yloc])  # [K, 14]
            wxs.append(WxS.T[xloc])  # [K, 14]
        if lvls:
            lvl_a = np.concatenate(lvls)
            y_a = np.concatenate(ys_l)
            x_a = np.concatenate(xs_l)
            wy_a = np.concatenate(wys, axis=0).astype(np.float32)
            wx_a = np.concatenate(wxs, axis=0).astype(np.float32)
        else:
            lvl_a = np.zeros(1, np.int64)
            y_a = np.zeros(1, np.int64)
            x_a = np.zeros(1, np.int64)
            wy_a = np.zeros((1, 196 // P_OUT), np.float32)
            wx_a = np.zeros((1, 196 // P_OUT), np.float32)
        rois.append(
            dict(
                b=int(batch_idx[r]),
                lvl=lvl_a,
                y=y_a,
                x=x_a,
                wy=wy_a,
                wx=wx_a,
                k=lvl_a.size,
                nch=max(1, -(-lvl_a.size // CHUNK_K)),
            )
        )
    return rois


# ---------------------------------------------------------------- device graph
_GRAPH_CACHE = {}


GROUP = 8  # ROIs per DMA group


def _build_graph(schedule):
    """SPMD Bass graph: identical for all cores; schedule = tuple of per-slot
    chunk counts (shared across cores).  Slots are processed in groups of
    GROUP; each group does one reg load (sync/SP HWDGE ring), one w2 load
    (scalar/ACT HWDGE ring) and one output store (gpsimd SWDGE), keeping DMA
    descriptor-generation off the critical path."""
    key = tuple(schedule)
    if key in _GRAPH_CACHE:
        return _GRAPH_CACHE[key]

    import concourse.bass as bass
    import concourse.bacc as bacc
    import concourse.tile as tile
    import concourse.mybir as mybir

    BF16 = mybir.dt.bfloat16
    F32 = mybir.dt.float32

    tot = int(sum(schedule))
    n_slots = len(schedule)
    assert n_slots % GROUP == 0
    n_groups = n_slots // GROUP

    nc = bacc.Bacc("TRN2", target_bir_lowering=False, debug=False)
    regs = nc.declare_dram_parameter("regs", [128, tot, 256], BF16, isOutput=False)
    wyxs = nc.declare_dram_parameter("wyxs", [128, tot, 28], BF16, isOutput=False)
    out = nc.declare_dram_parameter(
        "out", [n_groups, 98, GROUP * 512], BF16, isOutput=True
    )

    with tile.TileContext(nc) as tc:
        with (
            tc.tile_pool(name="data", bufs=4) as data_pool,
            tc.tile_pool(name="w2p", bufs=8) as w2_pool,
            tc.tile_pool(name="psum", bufs=4, space="PSUM") as psum_pool,
            tc.tile_pool(name="outp", bufs=3) as out_pool,
        ):
            start = 0
            for g in range(n_groups):
                ccs = schedule[g * GROUP : (g + 1) * GROUP]
                gcc = int(sum(ccs))
                reg_t = data_pool.tile([128, gcc * 256], BF16, tag="reg")
                wyx_t = data_pool.tile([128, gcc * 28], BF16, tag="wyx")
                # reg load split for finer-grained compute overlap (4-way for
                # the first group so compute starts as early as possible)
                nsplit = 4 if g == 0 else 2
                bounds = [
                    int(sum(ccs[: (i * GROUP) // nsplit])) for i in range(nsplit)
                ] + [gcc]
                for a, b in zip(bounds, bounds[1:]):
                    if a == b:
                        continue
                    nc.sync.dma_start(
                        reg_t[:, a * 256 : b * 256],
                        regs[:, start + a : start + b, :].rearrange(
                            "p t c -> p (t c)"
                        ),
                    )
                nc.scalar.dma_start(
                    wyx_t[:],
                    wyxs[:, start : start + gcc, :].rearrange("p t c -> p (t c)"),
                )
                ot = out_pool.tile([98, GROUP * 512], BF16)
                t0 = 0
                for i, cc in enumerate(ccs):
                    # one batched outer-product per ROI:
                    # w2[p, t, py*14+px] = wy[p, t, py] * wx[p, t, px]
                    w2_t = w2_pool.tile([128, cc * 196], BF16, tag="w2")
                    wyx_r = wyx_t[:, t0 * 28 : (t0 + cc) * 28].rearrange(
                        "p (t z) -> p t z", t=cc
                    )
                    # small ROIs' outer products go to the otherwise-idle
                    # gpsimd engine (~3x slower per element but off the DVE
                    # critical path); big ones to the DVE. A few cc==3 slots
                    # also go to gpsimd to equalize engine busy time.
                    to_gp = cc <= 2 or (cc == 3 and (g * GROUP + i) % 5 == 0)
                    tt_eng = nc.gpsimd if to_gp else nc.vector
                    tt_eng.tensor_mul(
                        w2_t[:].rearrange("p (t a b) -> p t a b", t=cc, a=P_OUT),
                        wyx_r[:, :, 0:14][:, :, :, None].broadcast_to(
                            [128, cc, P_OUT, P_OUT]
                        ),
                        wyx_r[:, :, 14:28][:, :, None, :].broadcast_to(
                            [128, cc, P_OUT, P_OUT]
                        ),
                    )
                    # two M-halves must live in different PSUM banks: a
                    # matmul's start=True clears the whole bank, so
                    # interleaved accumulation chains sharing a bank corrupt
                    # each other.
                    ps = psum_pool.tile([98, 1024], F32)
                    for t in range(cc):
                        for m in range(2):
                            nc.tensor.matmul(
                                ps[:, m * 512 : m * 512 + 256],
                                w2_t[:, t * 196 + m * 98 : t * 196 + (m + 1) * 98],
                                reg_t[:, (t0 + t) * 256 : (t0 + t + 1) * 256],
                                start=(t == 0),
                                stop=(t == cc - 1),
                            )
                    ps_view = ps[:].rearrange("p (m z) -> p m z", m=2)[:, :, 0:256]
                    ot_view = ot[:, i * 512 : (i + 1) * 512].rearrange(
                        "p (m c) -> p m c", m=2
                    )
                    nc.scalar.copy(ot_view, ps_view)
                    t0 += cc
                # tail stores go on the sync HWDGE ring (idle once loads are
                # done); earlier stores on gpsimd to keep the load ring clear
                if g >= n_groups - 2:
                    nc.sync.dma_start(out[g][:, : GROUP * 256], ot[:, : GROUP * 256])
                    nc.sync.dma_start(out[g][:, GROUP * 256 :], ot[:, GROUP * 256 :])
                else:
                    nc.gpsimd.dma_start(out[g], ot[:])
                start += gcc
    nc.compile()
    _GRAPH_CACHE[key] = nc
    return nc


# ---------------------------------------------------------------- entry point
def _run(feature_f4, feature_f8, boxes, batch_idx, trace=False):
    from concourse.bass_utils import run_bass_kernel_spmd

    feature_f4 = np.ascontiguousarray(np.asarray(feature_f4, dtype=np.float32))
    feature_f8 = np.ascontiguousarray(np.asarray(feature_f8, dtype=np.float32))
    boxes = np.asarray(boxes, dtype=np.float32)
    batch_idx = np.asarray(batch_idx)

    rois = _build_rois(boxes, batch_idx)
    R = len(rois)
    assert R % N_CORES == 0
    n_slots = R // N_CORES

    # shard: descending chunk count, round-robin deal
    order = sorted(range(R), key=lambda r: (-rois[r]["nch"], -rois[r]["k"], r))
    assign = [order[c::N_CORES] for c in range(N_CORES)]  # [core][slot] -> roi
    # snake-reorder slots so each GROUP of consecutive slots mixes large and
    # small ROIs (balanced DMA group sizes)
    n_groups = n_slots // GROUP
    slot_order = [g + n_groups * i for g in range(n_groups) for i in range(GROUP)]
    # then order groups small-load first (fast pipeline ramp-in) and
    # second-smallest last (fast drain)
    gw = []
    for g in range(n_groups):
        members = slot_order[g * GROUP : (g + 1) * GROUP]
        gw.append((sum(rois[assign[0][s]]["nch"] for s in members), g))
    asc = [g for _, g in sorted(gw)]
    g_order = asc[0::2] + asc[1::2][::-1]
    slot_order = [
        s for g in g_order for s in slot_order[g * GROUP : (g + 1) * GROUP]
    ]
    assign = [[a[s] for s in slot_order] for a in assign]
    schedule = [
        max(rois[assign[c][j]]["nch"] for c in range(N_CORES))
        for j in range(n_slots)
    ]
    tot = int(sum(schedule))

    # NHWC bf16 feature copies for row gathering
    feats_bf = [
        np.ascontiguousarray(feature_f4.transpose(0, 2, 3, 1)).astype(BF16_NP),
        np.ascontiguousarray(feature_f8.transpose(0, 2, 3, 1)).astype(BF16_NP),
    ]

    in_maps = []
    for c in range(N_CORES):
        regs_c = np.zeros((128, tot, 256), BF16_NP)
        wyxs_c = np.zeros((128, tot, 28), BF16_NP)
        startch = 0
        for j in range(n_slots):
            d = rois[assign[c][j]]
            k = d["k"]
            vals = np.empty((k, 256), BF16_NP)
            for lvl in (0, 1):
                sel = d["lvl"] == lvl
                if sel.any():
                    vals[sel] = feats_bf[lvl][d["b"]][d["y"][sel], d["x"][sel]]
            wy_bf = d["wy"].astype(BF16_NP)
            wx_bf = d["wx"].astype(BF16_NP)
            for t in range(schedule[j]):
                a, b = t * CHUNK_K, min((t + 1) * CHUNK_K, k)
                if a >= k:
                    break
                regs_c[0 : b - a, startch + t, :] = vals[a:b]
                wyxs_c[0 : b - a, startch + t, 0:14] = wy_bf[a:b]
                wyxs_c[0 : b - a, startch + t, 14:28] = wx_bf[a:b]
            startch += schedule[j]
        in_maps.append({"regs": regs_c, "wyxs": wyxs_c})

    nc = _build_graph(schedule)
    res = run_bass_kernel_spmd(
        nc, in_maps, core_ids=list(range(N_CORES)), trace=trace
    )

    # unshard + layout fix
    out_full = np.empty((R, 256, P_OUT, P_OUT), np.float32)
    n_groups = n_slots // GROUP
    for c in range(N_CORES):
        o = res.results[c]["out"].astype(np.float32)  # [n_groups, 98, GROUP*512]
        o = o.reshape(n_groups, 98, GROUP, 2, 256).transpose(0, 2, 3, 1, 4)
        o = o.reshape(n_slots, 196, 256).transpose(0, 2, 1)  # [slots, 256, 196]
        out_full[assign[c]] = o.reshape(n_slots, 256, P_OUT, P_OUT)
    return out_full, res


def kernel(feature_f4, feature_f8, boxes, batch_idx):
    out, _ = _run(feature_f4, feature_f8, boxes, batch_idx, trace=False)
    return out
